# revision 20
# baseline (speedup 1.0000x reference)
"""Label-smoothing KLDiv loss (batchmean) on 8 Trainium2 NeuronCores.

Math: with fv = SMOOTHING/(V-K), lv = (1-SMOOTHING)/K, and per-row unique
label sets L_b (|L_b| = U_b), the reference loss decomposes exactly as

  loss * B = C - fv * S - (lv - fv) * G
  C = sum_b [ U_b*lv*ln(lv) + (V-U_b)*fv*ln(fv) ]     (host, closed form)
  S = sum_{b,v} output[b,v]                           (device, 412MB reduction)
  G = sum_b sum_{v in L_b} output[b,v]                (device, indirect gather)

Each core streams a 256-row batch shard (51.5MB) through SBUF and reduces
it on the vector engine with scalar_tensor_tensor over tile PAIRS
(out=(a+0)+b with accum_out) — two SBUF reads per cycle, so the DVE runs
at 2 elem/cycle/lane and stays off the DMA critical path.  Tile sizes
taper geometrically at the end of the stream so the trailing reduction
after the final DMA lands is under two microseconds.
The 1280 label logits are gathered with ten per-column
indirect DMAs (the indirect engine consumes ONE offset per partition and
copies a contiguous run, so each gathered element needs its own column).
The host combines partial S/G in float64 with the closed-form C.

The shard is padded with 256 zeros: a global sum doesn't care how the
flat array splits across partitions, and duplicate labels in a row gather
a padded zero instead of needing a mask multiply on device.
"""

import math
from contextlib import ExitStack

import numpy as np

import concourse.bass as bass
import concourse.mybir as mybir
from concourse.bass_utils import run_bass_kernel_spmd

B = 2048
V = 50257
K = 5
NCORES = 8
SMOOTHING = 0.1

RPC = B // NCORES          # rows per core: 256
NFLAT = RPC * V            # 12,865,792 data elems per core
PAD = 256
NTOT = NFLAT + PAD         # 12,866,048 = 128 * 100,516
P = 128
FPP = NTOT // P            # 100,516 elems per partition
F_BIG = 6450               # 25.8KB/partition per big tile
NBIG = 14                  # big tiles cycle through 4 recycled slots
# Geometric taper (~2.3x) at the end: each pair's STT latency stays under
# the next pair's DMA time, so the post-stream tail is just the last tiny
# STT plus the final reduces (~2us) instead of a full big-pair STT.  The
# taper tiles get DEDICATED slots (41KB/partition total) so their DMAs
# never wait on the vector engine — the stream runs back-to-back.
TAPER = [2994] * 2 + [1302] * 2 + [566] * 2 + [246] * 2
SPANS = [F_BIG] * NBIG + TAPER
assert sum(SPANS) == FPP
NPAIR = len(SPANS) // 2    # 11 STT pair-reductions
NBUF = 4                   # 4 recycled big slots = 2 big pairs in flight
NG = (RPC * K) // P        # gather columns: 10

F32 = mybir.dt.float32
I32 = mybir.dt.int32

_CACHE: dict = {}


def build_module() -> bass.Bass:
    nc = bass.Bass()
    x = nc.dram_tensor("x", [NTOT], F32, kind="ExternalInput")
    gidx = nc.dram_tensor("gidx", [P, NG], I32, kind="ExternalInput")
    # NPAIR per-partition pair sums + 1 per-partition gather sum; the
    # final (trivial) summation over partitions/columns happens on host.
    res = nc.dram_tensor("res", [P, NPAIR + 1], F32, kind="ExternalOutput")

    x_flat = x[:]
    x2d = x_flat.rearrange("(p f) -> p f", p=P)
    xcol = x_flat.rearrange("(n one) -> n one", one=1)  # [NTOT, 1] gather view

    offs = [sum(SPANS[:t]) for t in range(len(SPANS))]
    add = mybir.AluOpType.add

    # Raw-bass program: this toolchain's walrus rejects instructions with
    # more than one semaphore wait, so every instruction below carries at
    # most one.  v_sem counts finished pair-reductions (slot recycling);
    # f_sem gates the single result store.
    with ExitStack() as ctx:
        xts = [
            ctx.enter_context(nc.sbuf_tensor(f"xt{i}", [P, F_BIG], F32))
            for i in range(NBUF)
        ]
        sts = [
            ctx.enter_context(nc.sbuf_tensor(f"st{i}", [P, fl], F32))
            for i, fl in enumerate(TAPER)
        ]
        idx_sb = ctx.enter_context(nc.sbuf_tensor([P, NG], I32))
        g_sb = ctx.enter_context(nc.sbuf_tensor([P, NG], F32))
        acc = ctx.enter_context(nc.sbuf_tensor([P, NPAIR + 1], F32))
        pair_sems = [
            ctx.enter_context(nc.semaphore(f"ps{i}")) for i in range(NPAIR)
        ]
        o_sem = ctx.enter_context(nc.semaphore("o_sem"))
        gi_sem = ctx.enter_context(nc.semaphore("gi_sem"))
        gg_sem = ctx.enter_context(nc.semaphore("gg_sem"))
        v_sem = ctx.enter_context(nc.semaphore("v_sem"))
        f_sem = ctx.enter_context(nc.semaphore("f_sem"))
        block = ctx.enter_context(nc.Block())

        @block.sync
        def _(sync):
            # Stream the shard; a big slot pair recycles once its STT
            # finished, taper tiles have their own slots (no wait).
            for t, fl in enumerate(SPANS):
                k = t // 2
                if t < NBIG:
                    if t >= NBUF:
                        sync.wait_ge(v_sem, k - 1)
                    dst = xts[t % NBUF][:, :fl]
                else:
                    dst = sts[t - NBIG][:, :]
                sync.dma_start(
                    out=dst, in_=x2d[:, offs[t] : offs[t] + fl]
                ).then_inc(pair_sems[k], 16)
            sync.wait_ge(f_sem, 1)
            sync.dma_start(out=res[:], in_=acc[:]).then_inc(o_sem, 16)

        @block.gpsimd
        def _(gpsimd):
            gpsimd.dma_start(out=idx_sb[:], in_=gidx[:]).then_inc(gi_sem, 16)
            gpsimd.wait_ge(gi_sem, 16)
            # The indirect engine reads ONE offset per partition and copies
            # a contiguous run from it, so gather column-by-column: each
            # DMA fetches one scattered fp32 per partition.  Duplicate
            # labels point at pad zeros.
            for j in range(NG):
                gpsimd.indirect_dma_start(
                    out=g_sb[:, j : j + 1],
                    out_offset=None,
                    in_=xcol,
                    in_offset=bass.IndirectOffsetOnAxis(
                        ap=idx_sb[:, j : j + 1], axis=0
                    ),
                ).then_inc(gg_sem, 16)

        @block.vector
        def _(vector):
            # Both tiles of a pair bump that pair's own semaphore, so one
            # wait covers both DMAs and out-of-order completion across
            # pairs (small taper tiles can beat earlier big tiles) is safe.
            for k in range(NPAIR):
                fl = SPANS[2 * k]
                if 2 * k < NBIG:
                    sl = xts[(2 * k) % NBUF][:, :fl]
                    sr = xts[(2 * k) % NBUF + 1][:, :fl]
                else:
                    sl = sts[2 * k - NBIG][:, :]
                    sr = sts[2 * k - NBIG + 1][:, :]
                vector.wait_ge(pair_sems[k], 32)
                vector.scalar_tensor_tensor(
                    out=sl,
                    in0=sl,
                    scalar=0.0,
                    in1=sr,
                    op0=add,
                    op1=add,
                    accum_out=acc[:, k : k + 1],
                ).then_inc(v_sem, 1)
            # Gathers finished long ago (they run under the stream); this
            # reduce lands the per-partition gather sum in acc's last
            # column, then the whole acc block stores to DRAM.
            vector.wait_ge(gg_sem, 16 * NG)
            vector.reduce_sum(
                out=acc[:, NPAIR : NPAIR + 1],
                in_=g_sb[:, :],
                axis=mybir.AxisListType.X,
            ).then_inc(f_sem, 1)

    return nc


def get_nc() -> bass.Bass:
    if "nc" not in _CACHE:
        _CACHE["nc"] = build_module()
    return _CACHE["nc"]


def prepare_in_maps(output: np.ndarray, labels: np.ndarray):
    """Shard batch across cores; flat gather indices with duplicate labels
    redirected to the zero pad (so they count once, matching .at[].set)."""
    output = np.ascontiguousarray(np.asarray(output, dtype=np.float32))
    lab = np.asarray(labels).astype(np.int64)

    first = np.ones((B, K), dtype=bool)
    for k in range(1, K):
        first[:, k] = ~(lab[:, k : k + 1] == lab[:, :k]).any(axis=1)
    u_total = float(first.sum())

    pad = np.zeros(PAD, dtype=np.float32)
    in_maps = []
    for c in range(NCORES):
        rows = slice(c * RPC, (c + 1) * RPC)
        shard = np.concatenate([output[rows].reshape(-1), pad])
        local_b = np.arange(RPC, dtype=np.int64)[:, None]
        flat_idx = local_b * V + lab[rows]
        flat_idx[~first[rows]] = NFLAT  # first pad element == 0.0
        in_maps.append(
            {"x": shard, "gidx": flat_idx.reshape(P, NG).astype(np.int32)}
        )
    return in_maps, u_total


def combine(results, u_total: float) -> np.ndarray:
    s_total = sum(
        float(r["res"][:, :NPAIR].astype(np.float64).sum()) for r in results
    )
    g_total = sum(
        float(r["res"][:, NPAIR].astype(np.float64).sum()) for r in results
    )
    fv = float(np.float32(SMOOTHING / (V - K)))
    lv = float(np.float32((1.0 - SMOOTHING) / K))
    c_term = u_total * lv * math.log(lv) + (B * V - u_total) * fv * math.log(fv)
    loss = (c_term - fv * s_total - (lv - fv) * g_total) / B
    return np.array(loss, dtype=np.float32)


def kernel(output: np.ndarray, labels: np.ndarray) -> np.ndarray:
    in_maps, u_total = prepare_in_maps(output, labels)
    results = run_bass_kernel_spmd(
        get_nc(), in_maps, core_ids=list(range(NCORES))
    ).results
    return combine(results, u_total)


# revision 21
# speedup vs baseline: 1.0161x; 1.0161x over previous
"""Label-smoothing KLDiv loss (batchmean) on 8 Trainium2 NeuronCores.

Math: with fv = SMOOTHING/(V-K), lv = (1-SMOOTHING)/K, and per-row unique
label sets L_b (|L_b| = U_b), the reference loss decomposes exactly as

  loss * B = C - fv * S - (lv - fv) * G
  C = sum_b [ U_b*lv*ln(lv) + (V-U_b)*fv*ln(fv) ]     (host, closed form)
  S = sum_{b,v} output[b,v]                           (device, 412MB reduction)
  G = sum_b sum_{v in L_b} output[b,v]                (device, indirect gather)

Each core streams a 256-row batch shard (51.5MB) through SBUF and reduces
it on the vector engine with scalar_tensor_tensor over tile PAIRS
(out=(a+0)+b with accum_out) — two SBUF reads per cycle, so the DVE runs
at 2 elem/cycle/lane and stays off the DMA critical path.  Tile sizes
taper geometrically at the end of the stream so the trailing reduction
after the final DMA lands is under two microseconds.
The 1280 label logits are gathered with ten per-column
indirect DMAs (the indirect engine consumes ONE offset per partition and
copies a contiguous run, so each gathered element needs its own column).
The host combines partial S/G in float64 with the closed-form C.

The shard is padded with 256 zeros: a global sum doesn't care how the
flat array splits across partitions, and duplicate labels in a row gather
a padded zero instead of needing a mask multiply on device.
"""

import math
from contextlib import ExitStack

import numpy as np

import concourse.bass as bass
import concourse.mybir as mybir
from concourse.bass_utils import run_bass_kernel_spmd

B = 2048
V = 50257
K = 5
NCORES = 8
SMOOTHING = 0.1

RPC = B // NCORES          # rows per core: 256
NFLAT = RPC * V            # 12,865,792 data elems per core
PAD = 256
NTOT = NFLAT + PAD         # 12,866,048 = 128 * 100,516
P = 128
FPP = NTOT // P            # 100,516 elems per partition
F_BIG = 8600               # 34.4KB/partition per big tile (DMA-efficient)
NBIG = 10                  # big tiles cycle through 4 recycled slots
# Geometric taper (~2.2x) at the end: each pair's STT latency stays under
# the next pair's DMA time, so the post-stream tail is just the last tiny
# STT plus the final reduce (~1.5us) instead of a full big-pair STT.  The
# taper tiles get DEDICATED slots (58KB/partition total) so their DMAs
# never wait on the vector engine — the stream runs back-to-back.
TAPER = [3900] * 2 + [1850] * 2 + [900] * 2 + [608] * 2
SPANS = [F_BIG] * NBIG + TAPER
assert sum(SPANS) == FPP
NPAIR = len(SPANS) // 2    # 11 STT pair-reductions
NBUF = 4                   # 4 recycled big slots = 2 big pairs in flight
NG = (RPC * K) // P        # gather columns: 10

F32 = mybir.dt.float32
I32 = mybir.dt.int32

_CACHE: dict = {}


def build_module() -> bass.Bass:
    nc = bass.Bass()
    x = nc.dram_tensor("x", [NTOT], F32, kind="ExternalInput")
    gidx = nc.dram_tensor("gidx", [P, NG], I32, kind="ExternalInput")
    # NPAIR per-partition pair sums + 1 per-partition gather sum; the
    # final (trivial) summation over partitions/columns happens on host.
    res = nc.dram_tensor("res", [P, NPAIR + 1], F32, kind="ExternalOutput")

    x_flat = x[:]
    x2d = x_flat.rearrange("(p f) -> p f", p=P)
    xcol = x_flat.rearrange("(n one) -> n one", one=1)  # [NTOT, 1] gather view

    offs = [sum(SPANS[:t]) for t in range(len(SPANS))]
    add = mybir.AluOpType.add

    # Raw-bass program: this toolchain's walrus rejects instructions with
    # more than one semaphore wait, so every instruction below carries at
    # most one.  v_sem counts finished pair-reductions (slot recycling);
    # f_sem gates the single result store.
    with ExitStack() as ctx:
        xts = [
            ctx.enter_context(nc.sbuf_tensor(f"xt{i}", [P, F_BIG], F32))
            for i in range(NBUF)
        ]
        sts = [
            ctx.enter_context(nc.sbuf_tensor(f"st{i}", [P, fl], F32))
            for i, fl in enumerate(TAPER)
        ]
        idx_sb = ctx.enter_context(nc.sbuf_tensor([P, NG], I32))
        g_sb = ctx.enter_context(nc.sbuf_tensor([P, NG], F32))
        acc = ctx.enter_context(nc.sbuf_tensor([P, NPAIR + 1], F32))
        pair_sems = [
            ctx.enter_context(nc.semaphore(f"ps{i}")) for i in range(NPAIR)
        ]
        o_sem = ctx.enter_context(nc.semaphore("o_sem"))
        gi_sem = ctx.enter_context(nc.semaphore("gi_sem"))
        gg_sem = ctx.enter_context(nc.semaphore("gg_sem"))
        v_sem = ctx.enter_context(nc.semaphore("v_sem"))
        f_sem = ctx.enter_context(nc.semaphore("f_sem"))
        block = ctx.enter_context(nc.Block())

        @block.sync
        def _(sync):
            # Stream the shard; a big slot pair recycles once its STT
            # finished, taper tiles have their own slots (no wait).
            for t, fl in enumerate(SPANS):
                k = t // 2
                if t < NBIG:
                    if t >= NBUF:
                        sync.wait_ge(v_sem, k - 1)
                    dst = xts[t % NBUF][:, :fl]
                else:
                    dst = sts[t - NBIG][:, :]
                sync.dma_start(
                    out=dst, in_=x2d[:, offs[t] : offs[t] + fl]
                ).then_inc(pair_sems[k], 16)
            sync.wait_ge(f_sem, 1)
            sync.dma_start(out=res[:], in_=acc[:]).then_inc(o_sem, 16)

        @block.gpsimd
        def _(gpsimd):
            gpsimd.dma_start(out=idx_sb[:], in_=gidx[:]).then_inc(gi_sem, 16)
            gpsimd.wait_ge(gi_sem, 16)
            # The indirect engine reads ONE offset per partition and copies
            # a contiguous run from it, so gather column-by-column: each
            # DMA fetches one scattered fp32 per partition.  Duplicate
            # labels point at pad zeros.
            for j in range(NG):
                gpsimd.indirect_dma_start(
                    out=g_sb[:, j : j + 1],
                    out_offset=None,
                    in_=xcol,
                    in_offset=bass.IndirectOffsetOnAxis(
                        ap=idx_sb[:, j : j + 1], axis=0
                    ),
                ).then_inc(gg_sem, 16)

        @block.vector
        def _(vector):
            # Both tiles of a pair bump that pair's own semaphore, so one
            # wait covers both DMAs and out-of-order completion across
            # pairs (small taper tiles can beat earlier big tiles) is safe.
            for k in range(NPAIR):
                fl = SPANS[2 * k]
                if 2 * k < NBIG:
                    sl = xts[(2 * k) % NBUF][:, :fl]
                    sr = xts[(2 * k) % NBUF + 1][:, :fl]
                else:
                    sl = sts[2 * k - NBIG][:, :]
                    sr = sts[2 * k - NBIG + 1][:, :]
                vector.wait_ge(pair_sems[k], 32)
                vector.scalar_tensor_tensor(
                    out=sl,
                    in0=sl,
                    scalar=0.0,
                    in1=sr,
                    op0=add,
                    op1=add,
                    accum_out=acc[:, k : k + 1],
                ).then_inc(v_sem, 1)
            # Gathers finished long ago (they run under the stream); this
            # reduce lands the per-partition gather sum in acc's last
            # column, then the whole acc block stores to DRAM.
            vector.wait_ge(gg_sem, 16 * NG)
            vector.reduce_sum(
                out=acc[:, NPAIR : NPAIR + 1],
                in_=g_sb[:, :],
                axis=mybir.AxisListType.X,
            ).then_inc(f_sem, 1)

    return nc


def get_nc() -> bass.Bass:
    if "nc" not in _CACHE:
        _CACHE["nc"] = build_module()
    return _CACHE["nc"]


def prepare_in_maps(output: np.ndarray, labels: np.ndarray):
    """Shard batch across cores; flat gather indices with duplicate labels
    redirected to the zero pad (so they count once, matching .at[].set)."""
    output = np.ascontiguousarray(np.asarray(output, dtype=np.float32))
    lab = np.asarray(labels).astype(np.int64)

    first = np.ones((B, K), dtype=bool)
    for k in range(1, K):
        first[:, k] = ~(lab[:, k : k + 1] == lab[:, :k]).any(axis=1)
    u_total = float(first.sum())

    pad = np.zeros(PAD, dtype=np.float32)
    in_maps = []
    for c in range(NCORES):
        rows = slice(c * RPC, (c + 1) * RPC)
        shard = np.concatenate([output[rows].reshape(-1), pad])
        local_b = np.arange(RPC, dtype=np.int64)[:, None]
        flat_idx = local_b * V + lab[rows]
        flat_idx[~first[rows]] = NFLAT  # first pad element == 0.0
        in_maps.append(
            {"x": shard, "gidx": flat_idx.reshape(P, NG).astype(np.int32)}
        )
    return in_maps, u_total


def combine(results, u_total: float) -> np.ndarray:
    s_total = sum(
        float(r["res"][:, :NPAIR].astype(np.float64).sum()) for r in results
    )
    g_total = sum(
        float(r["res"][:, NPAIR].astype(np.float64).sum()) for r in results
    )
    fv = float(np.float32(SMOOTHING / (V - K)))
    lv = float(np.float32((1.0 - SMOOTHING) / K))
    c_term = u_total * lv * math.log(lv) + (B * V - u_total) * fv * math.log(fv)
    loss = (c_term - fv * s_total - (lv - fv) * g_total) / B
    return np.array(loss, dtype=np.float32)


def kernel(output: np.ndarray, labels: np.ndarray) -> np.ndarray:
    in_maps, u_total = prepare_in_maps(output, labels)
    results = run_bass_kernel_spmd(
        get_nc(), in_maps, core_ids=list(range(NCORES))
    ).results
    return combine(results, u_total)


# revision 29
# speedup vs baseline: 1.0316x; 1.0152x over previous
"""Label-smoothing KLDiv loss (batchmean) on 8 Trainium2 NeuronCores.

Math: with fv = SMOOTHING/(V-K), lv = (1-SMOOTHING)/K, and per-row unique
label sets L_b (|L_b| = U_b), the reference loss decomposes exactly as

  loss * B = C - fv * S - (lv - fv) * G
  C = sum_b [ U_b*lv*ln(lv) + (V-U_b)*fv*ln(fv) ]     (host, closed form)
  S = sum_{b,v} output[b,v]                           (device, 412MB reduction)
  G = sum_b sum_{v in L_b} output[b,v]                (device, indirect gather)

Each core streams a 256-row batch shard (51.5MB) through SBUF and reduces
it on the vector engine with scalar_tensor_tensor over tile PAIRS
(out=(a+0)+b with accum_out) — two SBUF reads per cycle, so the DVE runs
at 2 elem/cycle/lane and stays off the DMA critical path.  Tile sizes
taper geometrically at the end of the stream so the trailing reduction
after the final DMA lands is under two microseconds.
The 1280 label logits are gathered with ten per-column
indirect DMAs (the indirect engine consumes ONE offset per partition and
copies a contiguous run, so each gathered element needs its own column).
The host combines partial S/G in float64 with the closed-form C.

The shard is padded with 256 zeros: a global sum doesn't care how the
flat array splits across partitions, and duplicate labels in a row gather
a padded zero instead of needing a mask multiply on device.
"""

import math
from contextlib import ExitStack

import numpy as np

import concourse.bass as bass
import concourse.mybir as mybir
from concourse.bass_utils import run_bass_kernel_spmd

B = 2048
V = 50257
K = 5
NCORES = 8
SMOOTHING = 0.1

RPC = B // NCORES          # rows per core: 256
NFLAT = RPC * V            # 12,865,792 data elems per core
PAD = 256
NTOT = NFLAT + PAD         # 12,866,048 = 128 * 100,516
P = 128
FPP = NTOT // P            # 100,516 elems per partition
F_BIG = 8600               # 34.4KB/partition per big tile (DMA-efficient)
NBIG = 10                  # big tiles cycle through 4 recycled slots
# Geometric taper (~2.2x) at the end: each pair's STT latency stays under
# the next pair's DMA time, so the post-stream tail is just the last tiny
# STT plus the final reduce (~1.5us) instead of a full big-pair STT.  The
# taper tiles get DEDICATED slots (58KB/partition total) so their DMAs
# never wait on the vector engine — the stream runs back-to-back.
TAPER = [3900] * 2 + [1850] * 2 + [900] * 2 + [608] * 2
SPANS = [F_BIG] * NBIG + TAPER
assert sum(SPANS) == FPP
NPAIR = len(SPANS) // 2    # 11 STT pair-reductions
NBUF = 4                   # 4 recycled big slots = 2 big pairs in flight
NG = (RPC * K) // P        # gather columns: 10

F32 = mybir.dt.float32
I32 = mybir.dt.int32

_CACHE: dict = {}


def build_module() -> bass.Bass:
    nc = bass.Bass()
    x = nc.dram_tensor("x", [NTOT], F32, kind="ExternalInput")
    gidx = nc.dram_tensor("gidx", [P, NG], I32, kind="ExternalInput")
    # NPAIR per-partition pair sums + 1 per-partition gather sum; the
    # final (trivial) summation over partitions/columns happens on host.
    res = nc.dram_tensor("res", [P, NPAIR + 1], F32, kind="ExternalOutput")

    x_flat = x[:]
    x2d = x_flat.rearrange("(p f) -> p f", p=P)
    xcol = x_flat.rearrange("(n one) -> n one", one=1)  # [NTOT, 1] gather view

    offs = [sum(SPANS[:t]) for t in range(len(SPANS))]
    add = mybir.AluOpType.add

    # Raw-bass program: this toolchain's walrus rejects instructions with
    # more than one semaphore wait, so every instruction below carries at
    # most one.  v_sem counts finished pair-reductions (slot recycling);
    # f_sem gates the single result store.
    with ExitStack() as ctx:
        xts = [
            ctx.enter_context(nc.sbuf_tensor(f"xt{i}", [P, F_BIG], F32))
            for i in range(NBUF)
        ]
        sts = [
            ctx.enter_context(nc.sbuf_tensor(f"st{i}", [P, fl], F32))
            for i, fl in enumerate(TAPER)
        ]
        idx_sb = ctx.enter_context(nc.sbuf_tensor([P, NG], I32))
        g_sb = ctx.enter_context(nc.sbuf_tensor([P, NG], F32))
        acc = ctx.enter_context(nc.sbuf_tensor([P, NPAIR + 1], F32))
        pair_sems = [
            ctx.enter_context(nc.semaphore(f"ps{i}")) for i in range(NPAIR)
        ]
        o_sem = ctx.enter_context(nc.semaphore("o_sem"))
        gi_sem = ctx.enter_context(nc.semaphore("gi_sem"))
        gg_sem = ctx.enter_context(nc.semaphore("gg_sem"))
        v_sem = ctx.enter_context(nc.semaphore("v_sem"))
        f_sem = ctx.enter_context(nc.semaphore("f_sem"))
        block = ctx.enter_context(nc.Block())

        @block.sync
        def _(sync):
            # Stream the shard; a big slot pair recycles once its STT
            # finished, taper tiles have their own slots (no wait).
            for t, fl in enumerate(SPANS):
                k = t // 2
                if t < NBIG:
                    if t >= NBUF:
                        sync.wait_ge(v_sem, k - 1)
                    dst = xts[t % NBUF][:, :fl]
                else:
                    dst = sts[t - NBIG][:, :]
                sync.dma_start(
                    out=dst, in_=x2d[:, offs[t] : offs[t] + fl]
                ).then_inc(pair_sems[k], 16)
            sync.wait_ge(f_sem, 1)
            sync.dma_start(out=res[:], in_=acc[:]).then_inc(o_sem, 16)

        @block.gpsimd
        def _(gpsimd):
            gpsimd.dma_start(out=idx_sb[:], in_=gidx[:]).then_inc(gi_sem, 16)
            gpsimd.wait_ge(gi_sem, 16)
            # The indirect engine reads ONE offset per partition and copies
            # a contiguous run from it, so gather column-by-column: each
            # DMA fetches one scattered fp32 per partition.  Duplicate
            # labels point at pad zeros.
            for j in range(NG):
                gpsimd.indirect_dma_start(
                    out=g_sb[:, j : j + 1],
                    out_offset=None,
                    in_=xcol,
                    in_offset=bass.IndirectOffsetOnAxis(
                        ap=idx_sb[:, j : j + 1], axis=0
                    ),
                ).then_inc(gg_sem, 16)

        @block.vector
        def _(vector):
            # Both tiles of a pair bump that pair's own semaphore, so one
            # wait covers both DMAs and out-of-order completion across
            # pairs (small taper tiles can beat earlier big tiles) is safe.
            for k in range(NPAIR):
                fl = SPANS[2 * k]
                if 2 * k < NBIG:
                    sl = xts[(2 * k) % NBUF][:, :fl]
                    sr = xts[(2 * k) % NBUF + 1][:, :fl]
                else:
                    sl = sts[2 * k - NBIG][:, :]
                    sr = sts[2 * k - NBIG + 1][:, :]
                vector.wait_ge(pair_sems[k], 32)
                vector.scalar_tensor_tensor(
                    out=sl,
                    in0=sl,
                    scalar=0.0,
                    in1=sr,
                    op0=add,
                    op1=add,
                    accum_out=acc[:, k : k + 1],
                ).then_inc(v_sem, 1)
            # Gathers finished long ago (they run under the stream); this
            # reduce lands the per-partition gather sum in acc's last
            # column, then the whole acc block stores to DRAM.
            vector.wait_ge(gg_sem, 16 * NG)
            vector.reduce_sum(
                out=acc[:, NPAIR : NPAIR + 1],
                in_=g_sb[:, :],
                axis=mybir.AxisListType.X,
            ).then_inc(f_sem, 1)

    return nc


def get_nc() -> bass.Bass:
    if "nc" not in _CACHE:
        _CACHE["nc"] = build_module()
    return _CACHE["nc"]


def prepare_in_maps(output: np.ndarray, labels: np.ndarray):
    """Shard batch across cores; flat gather indices with duplicate labels
    redirected to the zero pad (so they count once, matching .at[].set)."""
    output = np.ascontiguousarray(np.asarray(output, dtype=np.float32))
    lab = np.asarray(labels).astype(np.int64)

    first = np.ones((B, K), dtype=bool)
    for k in range(1, K):
        first[:, k] = ~(lab[:, k : k + 1] == lab[:, :k]).any(axis=1)
    u_total = float(first.sum())

    pad = np.zeros(PAD, dtype=np.float32)
    in_maps = []
    for c in range(NCORES):
        rows = slice(c * RPC, (c + 1) * RPC)
        shard = np.concatenate([output[rows].reshape(-1), pad])
        local_b = np.arange(RPC, dtype=np.int64)[:, None]
        flat_idx = local_b * V + lab[rows]
        flat_idx[~first[rows]] = NFLAT  # first pad element == 0.0
        in_maps.append(
            {"x": shard, "gidx": flat_idx.reshape(P, NG).astype(np.int32)}
        )
    return in_maps, u_total


def combine(results, u_total: float) -> np.ndarray:
    s_total = sum(
        float(r["res"][:, :NPAIR].astype(np.float64).sum()) for r in results
    )
    g_total = sum(
        float(r["res"][:, NPAIR].astype(np.float64).sum()) for r in results
    )
    fv = float(np.float32(SMOOTHING / (V - K)))
    lv = float(np.float32((1.0 - SMOOTHING) / K))
    c_term = u_total * lv * math.log(lv) + (B * V - u_total) * fv * math.log(fv)
    loss = (c_term - fv * s_total - (lv - fv) * g_total) / B
    return np.array(loss, dtype=np.float32)


def kernel(output: np.ndarray, labels: np.ndarray) -> np.ndarray:
    in_maps, u_total = prepare_in_maps(output, labels)
    results = run_bass_kernel_spmd(
        get_nc(), in_maps, core_ids=list(range(NCORES))
    ).results
    return combine(results, u_total)
